# revision 67
# baseline (speedup 1.0000x reference)
"""Trainium2 Bass kernel for nn_Block (dense transformer block), 8-core SPMD.

Sharding: core c -> batch c//2, half of the causal q-blocks (interleaved
assignment {i : i%4 in {0,3}} / {i%4 in {1,2}} for exact causal balance).
K/V are computed per-core for the whole batch (duplicated across the 2 cores
of a batch); everything after attention is purely token-parallel, so no
collectives are needed and each core writes its own output rows.

Layout strategy (all matmuls bf16, fp32 accumulation):
  - x^T, K^T, Q^T kept feature-on-partitions so attention scores are computed
    directly transposed: S^T[k,q] = (K^T chunk).T @ Q^T.
  - V carries 64 appended ones-columns (vo rows 64:127), so the O^T
    accumulation matmul broadcasts the softmax denominator across 64 PSUM
    partitions for free; the reciprocal then runs 64-wide on DVE.
  - Attention runs head-major: one [128,TOKS] PSUM accumulator per head.
  - FFN: all of h^T is produced first (bf16, 8MB), then the second linear
    accumulates the full 4096-deep contraction in PSUM per token block (no
    SBUF accumulator round-trips). b2 is folded into ln1_b on the host
    (b1 compensated by -b2 @ w1) so the finalize is one add + LN.
  - LayerNorm uses a fused scalar_tensor_tensor for (xc*rstd)*a and writes
    its output into the xc scratch tile when the caller allows.
"""

import numpy as np
import ml_dtypes

import concourse.bacc as bacc
import concourse.mybir as mybir
import concourse.tile as tile
from concourse.alu_op_type import AluOpType
from concourse.masks import make_identity
from concourse.bass_utils import run_bass_kernel_spmd

BF = mybir.dt.bfloat16
F32 = mybir.dt.float32
AF = mybir.ActivationFunctionType
AX = mybir.AxisListType
bf16 = ml_dtypes.bfloat16

EPS = 1e-5
NEG = -1e30


class Cfg:
    def __init__(self, ne=1024, sl=2048, nh=16, nhid=4096, bs=4):
        self.ne, self.sl, self.nh, self.nhid, self.bs = ne, sl, nh, nhid, bs
        self.dh = 64
        self.e = ne // 128          # feature chunks
        self.nb = sl // 128         # k/token blocks per batch
        self.slots = self.nb // 2   # q-blocks per core
        self.toks = self.slots * 128
        self.pairs = nh // 2
        self.fch = nhid // 128      # ffn feature chunks
        self.scale = self.dh ** -0.5


FULL = Cfg()


def blocks_for(par, cfg, causal):
    if causal:
        keep = (0, 3) if par == 0 else (1, 2)
        return [i for i in range(cfg.nb) if i % 4 in keep]
    return list(range(par * cfg.slots, (par + 1) * cfg.slots))


def chunks(start, end, step=512):
    out = []
    c = start
    while c < end:
        w = min(end, (c // step + 1) * step) - c
        out.append((c, w))
        c += w
    return out


def layer_norm(nc, pool, out_ap, x_ap, a_ap, b_ap, n, tag, eps_ap):
    """out = (x - mean(x)) / (std(x, ddof=1) + EPS) * a + b, rows on
    partitions. If out_ap is None, writes into the xc scratch tile and
    returns that AP (saves a pool buffer for fire-and-forget DMA sources)."""
    st = pool.tile([128, 8], F32, tag=f"{tag}s", name=f"{tag}s")
    nc.vector.reduce_sum(st[:, 0:1], x_ap, axis=AX.X)
    nc.scalar.mul(st[:, 1:2], st[:, 0:1], -1.0 / n)
    xc = pool.tile([128, n], F32, tag=f"{tag}xc", name=f"{tag}xc")
    nc.scalar.add(xc, x_ap, st[:, 1:2])
    tmp = pool.tile([128, n], F32, tag=f"{tag}t", name=f"{tag}t")
    nc.scalar.activation(tmp, xc, AF.Square, accum_out=st[:, 2:3])
    nc.scalar.activation(st[:, 3:4], st[:, 2:3], AF.Sqrt, scale=1.0 / (n - 1))
    nc.scalar.add(st[:, 4:5], st[:, 3:4], eps_ap)
    nc.vector.reciprocal(st[:, 5:6], st[:, 4:5])
    nc.vector.scalar_tensor_tensor(tmp, xc, st[:, 5:6], a_ap,
                                   AluOpType.mult, AluOpType.mult)
    if out_ap is None:
        out_ap = xc
    nc.vector.tensor_add(out_ap, tmp, b_ap)
    return out_ap


def emit(tc, cfg, io, causal, upto="full"):
    nc = tc.nc
    E, NB, SLOTS, PAIRS = cfg.e, cfg.nb, cfg.slots, cfg.pairs
    NE, SL, TOKS, FCH = cfg.ne, cfg.sl, cfg.toks, cfg.fch

    def vk_phase(xtp, qkvp, psk, psv, kt, qt, vo, kb, vb, qb):
        xt = xtp.tile([128, E, SL], BF, tag="xt", name="xt", bufs=1)
        xt_src = io["xt"].rearrange("(e p) t -> p e t", p=128)
        vw = qkvp.tile([128, E, NE], BF, tag="w", name="w")
        vw_src = io["vw"].rearrange("(e p) n -> p e n", p=128)
        # interleave, with a small leading slice of xt so the first V psum
        # chains can start as soon as possible
        for e in range(E):
            nc.sync.dma_start(vw[:, e, :], vw_src[:, e, :])
            nc.sync.dma_start(xt[:, e, 0:256], xt_src[:, e, 0:256])
        for e in range(E):
            nc.sync.dma_start(xt[:, e, 256:], xt_src[:, e, 256:])
        nc.gpsimd.memset(vo[:, :, :, 64:128], 1.0)
        for j in range(NB):
            ps = psv.tile([128, NE], F32, tag="pv", name="pv")
            for (cs, cw) in chunks(0, NE):
                for e in range(E):
                    nc.tensor.matmul(
                        ps[:, cs:cs + cw],
                        lhsT=xt[:, e, j * 128:(j + 1) * 128],
                        rhs=vw[:, e, cs:cs + cw],
                        start=(e == 0), stop=(e == E - 1))
            nc.vector.tensor_add(
                vo[:, j, :, 0:64],
                ps.rearrange("p (h d) -> p h d", d=64),
                vb.rearrange("p (h d) -> p h d", d=64))
        # K^T all pairs
        kw = qkvp.tile([128, E, NE], BF, tag="w", name="w")
        kw_src = io["kw"].rearrange("(e p) n -> p e n", p=128)
        for e in range(E):
            nc.sync.dma_start(kw[:, e, :], kw_src[:, e, :])
        for pair in range(PAIRS):
            for (cs, cw) in chunks(0, SL, 1024):
                ps = psk.tile([128, 1024], F32, tag="pk", name="pk")[:, :cw]
                for (c2, w2) in chunks(0, cw):
                    for e in range(E):
                        nc.tensor.matmul(
                            ps[:, c2:c2 + w2],
                            lhsT=kw[:, e, pair * 128:(pair + 1) * 128],
                            rhs=xt[:, e, cs + c2:cs + c2 + w2],
                            start=(e == 0), stop=(e == E - 1))
                nc.scalar.activation(kt[:, pair, cs:cs + cw], ps,
                                     AF.Identity, bias=kb[:, pair:pair + 1])
    def q_phase(qp, psk, qt, qb):
        qw = qp.tile([128, E, NE], BF, tag="qw", name="qw", bufs=1)
        qw_src = io["qw"].rearrange("(e p) n -> p e n", p=128)
        for e in range(E):
            nc.sync.dma_start(qw[:, e, :], qw_src[:, e, :])
        xqt = qp.tile([128, E, TOKS], BF, tag="xqt", name="xqt", bufs=1)
        xqt_src = io["xqt"].rearrange("(e p) t -> p e t", p=128)
        for e in range(E):
            nc.sync.dma_start(xqt[:, e, :], xqt_src[:, e, :])
        for pair in range(PAIRS):
            ps = psk.tile([128, 1024], F32, tag="pk", name="pk")[:, :TOKS]
            for (cs, cw) in chunks(0, TOKS):
                for e in range(E):
                    nc.tensor.matmul(
                        ps[:, cs:cs + cw],
                        lhsT=qw[:, e, pair * 128:(pair + 1) * 128],
                        rhs=xqt[:, e, cs:cs + cw],
                        start=(e == 0), stop=(e == E - 1))
            nc.vector.tensor_scalar_add(qt[:, pair, :], ps,
                                        qb[:, pair:pair + 1])

    def att_head(attp, psa1, psa2, kt, qt, vo, yt, am, pair, base):
        head = 2 * pair + (base >> 6)
        po = psa1.tile([128, TOKS], F32, tag="pso", name="pso")
        pts = {}

        def emit_scores(j):
            c0 = (j // 2) * 128 if causal else 0
            pt = attp.tile([128, TOKS], BF, tag="pt", name="pt", bufs=6)
            pts[j] = pt
            pss = psa2.tile([128, TOKS], F32, tag="pss",
                            name="pss")[:, :TOKS - c0]
            for (cs, cw) in chunks(0, TOKS - c0):
                nc.tensor.matmul(
                    pss[:, cs:cs + cw],
                    lhsT=kt[base:base + 64, pair, j * 128:(j + 1) * 128],
                    rhs=qt[base:base + 64, pair, c0 + cs:c0 + cs + cw],
                    start=True, stop=True)
            if not causal:
                amj = attp.tile([128, TOKS], F32, tag="amj", name="amj")
                nc.sync.dma_start(
                    amj, io["amask_full"].rearrange("j p q -> p j q")[:, j, :])
                nc.vector.tensor_add(pss, pss, amj)
            nc.scalar.activation(pt[:, c0:], pss, AF.Exp, scale=cfg.scale)
            if causal:
                nc.vector.tensor_mul(
                    pt[:, c0:c0 + 128], pt[:, c0:c0 + 128], am[:, j, :])

        def emit_av(j):
            c0 = (j // 2) * 128 if causal else 0
            pt = pts.pop(j)
            for (cs, cw) in chunks(c0, TOKS):
                if causal:
                    stop_j = 2 * (min((cs // 512 + 1) * 4, SLOTS) - 1) + 1
                else:
                    stop_j = NB - 1
                nc.tensor.matmul(
                    po[:, cs:cs + cw], lhsT=vo[:, j, head, :],
                    rhs=pt[:, cs:cs + cw], start=(j == 0),
                    stop=(j == stop_j))

        # engines run their queues in program order, so software-pipeline the
        # emission: scores run SKEW iterations ahead of the AV consumer, and
        # PE never sits behind an AV waiting for its exp
        SKEW = 3
        for j in range(NB + SKEW):
            if j < NB:
                emit_scores(j)
            if j - SKEW >= 0:
                emit_av(j - SKEW)
        # vo rows 64..127 are all-ones, so po[64:128] holds the softmax
        # denominator already broadcast across 64 partitions
        rbi = attp.tile([64, TOKS], F32, tag="rbi", name="rbi")
        nc.vector.reciprocal(rbi, po[64:128, :])
        if base == 0:
            nc.vector.tensor_mul(yt[0:64, pair, :], po[0:64, :], rbi)
        else:
            ystg = attp.tile([64, TOKS], BF, tag="ystg", name="ystg")
            nc.vector.tensor_mul(ystg, po[0:64, :], rbi)
            nc.sync.dma_start(yt[64:128, pair, :], ystg)

    def oproj_phase(op, pso_p, ptr_p, yt, x1, x1t, ident, eps_ap):
        ow = op.tile([128, E, NE], BF, tag="ow", name="ow", bufs=1)
        ow_src = io["ow"].rearrange("(f p) n -> p f n", p=128)
        for f in range(E):
            nc.sync.dma_start(ow[:, f, :], ow_src[:, f, :])
        ln1a = op.tile([128, NE], F32, tag="ln1a", name="ln1a", bufs=1)
        nc.sync.dma_start(ln1a, io["ln1a"])
        ln1b = op.tile([128, NE], F32, tag="ln1b", name="ln1b", bufs=1)
        nc.sync.dma_start(ln1b, io["ln1b"])
        xq_src = io["xq"].rearrange("(b p) n -> b p n", p=128)
        # transposes for tb are emitted two iterations late so PE does not
        # stall behind tb's LayerNorm before starting tb+1's o_proj matmuls
        for tb in range(SLOTS + 3):
            if tb < SLOTS:
                ps = pso_p.tile([128, NE], F32, tag="po", name="po")
                for (cs, cw) in chunks(0, NE):
                    for f in range(E):
                        nc.tensor.matmul(
                            ps[:, cs:cs + cw],
                            lhsT=yt[:, f, tb * 128:(tb + 1) * 128],
                            rhs=ow[:, f, cs:cs + cw],
                            start=(f == 0), stop=(f == E - 1))
                xq_t = op.tile([128, NE], F32, tag="xq", name="xq")
                nc.sync.dma_start(xq_t, xq_src[tb])
                t2 = op.tile([128, NE], F32, tag="t2", name="t2")
                nc.vector.tensor_add(t2, ps, xq_t)
                layer_norm(nc, op, x1[:, tb, :], t2, ln1a, ln1b, NE, "ln1",
                           eps_ap)
            if tb >= 3:
                for e in range(E):
                    ptr = ptr_p.tile([128, 128], BF, tag="ptr", name="ptr",
                                     bufs=2)
                    nc.tensor.transpose(
                        ptr, x1[:, tb - 3, e * 128:(e + 1) * 128], ident)
                    nc.scalar.copy(
                        x1t[:, e, (tb - 3) * 128:(tb - 2) * 128], ptr)

    def ffn_phase(fp, x1, x1t, eps_ap):
        b1c = fp.tile([128, FCH], F32, tag="b1c", name="b1c", bufs=1)
        nc.sync.dma_start(b1c, io["b1c"])
        ln2a = fp.tile([128, NE], F32, tag="ln2a", name="ln2a", bufs=1)
        nc.sync.dma_start(ln2a, io["ln2a"])
        ln2b = fp.tile([128, NE], F32, tag="ln2b", name="ln2b", bufs=1)
        nc.sync.dma_start(ln2b, io["ln2b"])
        w2t = fp.tile([128, FCH, NE], BF, tag="w2t", name="w2t", bufs=1)
        w2_src = io["w2"].rearrange("(f p) n -> p f n", p=128)
        ht = fp.tile([128, FCH, TOKS], BF, tag="ht", name="ht", bufs=1)
        with tc.tile_pool(name="psff1", bufs=3, space="PSUM") as ps1:
            for f in range(FCH):
                w1f = fp.tile([128, E, 128], BF, tag="w1f", name="w1f", bufs=3)
                nc.sync.dma_start(
                    w1f, io["w1p"][f].rearrange("(e p) q -> p e q", p=128))
                nc.sync.dma_start(w2t[:, f, :], w2_src[:, f, :])
                psh = ps1.tile([128, TOKS], F32, tag="psh", name="psh")
                for (cs, cw) in chunks(0, TOKS):
                    for e in range(E):
                        nc.tensor.matmul(
                            psh[:, cs:cs + cw], lhsT=w1f[:, e, :],
                            rhs=x1t[:, e, cs:cs + cw],
                            start=(e == 0), stop=(e == E - 1))
                nc.scalar.activation(ht[:, f, :], psh,
                                     AF.Relu, bias=b1c[:, f:f + 1])
        out_dst = io["out"].rearrange("(b p) n -> b p n", p=128)
        with tc.tile_pool(name="psff2", bufs=3, space="PSUM") as ps2:
            for tb in range(SLOTS):
                psF = ps2.tile([128, NE], F32, tag="psF", name="psF")
                for fi in range(FCH):
                    for (cs, cw) in chunks(0, NE):
                        nc.tensor.matmul(
                            psF[:, cs:cs + cw],
                            lhsT=ht[:, fi, tb * 128:(tb + 1) * 128],
                            rhs=w2t[:, fi, cs:cs + cw],
                            start=(fi == 0), stop=(fi == FCH - 1))
                t1 = fp.tile([128, NE], F32, tag="ft1", name="ft1")
                nc.vector.tensor_add(t1, psF, x1[:, tb, :])
                outt = layer_norm(nc, fp, None, t1, ln2a, ln2b, NE, "ln2",
                                  eps_ap)
                nc.sync.dma_start(out_dst[tb], outt)

    with tc.tile_pool(name="const", bufs=1) as constp:
        ident = constp.tile([128, 128], BF, tag="ident", name="ident")
        make_identity(nc, ident)
        eps_ap = constp.tile([128, 1], F32, tag="eps", name="eps")
        nc.vector.memset(eps_ap, EPS)
        qb = constp.tile([128, PAIRS], F32, tag="qb", name="qb")
        nc.sync.dma_start(qb, io["qb"])
        kb = constp.tile([128, PAIRS], F32, tag="kb", name="kb")
        nc.sync.dma_start(kb, io["kb"])
        vb = constp.tile([128, NE], F32, tag="vb", name="vb")
        nc.sync.dma_start(vb, io["vb"])
        am = None
        if causal:
            am = constp.tile([128, NB, 128], BF, tag="am", name="am")
            nc.sync.dma_start(am, io["amask"].rearrange("j p q -> p j q"))

        ytp_cm = tc.tile_pool(name="ytp", bufs=1)
        ytp = ytp_cm.__enter__()
        yt = ytp.tile([128, PAIRS, TOKS], BF, tag="yt", name="yt")

        with tc.tile_pool(name="kqvo", bufs=1) as kqvo:
            kt = kqvo.tile([128, PAIRS, SL], BF, tag="kt", name="kt")
            qt = kqvo.tile([128, PAIRS, TOKS], BF, tag="qt", name="qt")
            vo = kqvo.tile([128, NB, cfg.nh, 128], BF, tag="vo", name="vo")
            with tc.tile_pool(name="psk", bufs=2, space="PSUM") as psk:
                with (
                    tc.tile_pool(name="qkv", bufs=2) as qkvp,
                    tc.tile_pool(name="xtp", bufs=1) as xtp,
                    tc.tile_pool(name="psv", bufs=2, space="PSUM") as psv,
                ):
                    vk_phase(xtp, qkvp, psk, psv, kt, qt, vo, kb, vb, qb)
                with tc.tile_pool(name="qp", bufs=1) as qp:
                    q_phase(qp, psk, qt, qb)
            if upto != "qkv":
                with (
                    tc.tile_pool(name="att", bufs=3) as attp,
                    tc.tile_pool(name="psatt1", bufs=1, space="PSUM") as psa1,
                    tc.tile_pool(name="psatt2", bufs=3, space="PSUM") as psa2,
                ):
                    for pair in range(PAIRS):
                        for base in (0, 64):
                            att_head(attp, psa1, psa2, kt, qt, vo, yt, am,
                                     pair, base)

        x1p_cm = None
        if upto in ("oproj", "full"):
            x1p_cm = tc.tile_pool(name="x1p", bufs=1, side="right")
            x1p = x1p_cm.__enter__()
            x1 = x1p.tile([128, SLOTS, NE], BF, tag="x1", name="x1")
            x1t = x1p.tile([128, E, TOKS], BF, tag="x1t", name="x1t")
            with (
                tc.tile_pool(name="oproj", bufs=3) as op,
                tc.tile_pool(name="psop", bufs=3, space="PSUM") as pso_p,
                tc.tile_pool(name="psoptr", bufs=1, space="PSUM") as ptr_p,
            ):
                oproj_phase(op, pso_p, ptr_p, yt, x1, x1t, ident, eps_ap)

        ytp_cm.__exit__(None, None, None)

        if upto == "full":
            with tc.tile_pool(name="ffn", bufs=2) as fp:
                ffn_phase(fp, x1, x1t, eps_ap)
        else:
            dummy = constp.tile([128, PAIRS], F32, tag="dummy", name="dummy")
            nc.vector.tensor_copy(dummy, qb)
            nc.sync.dma_start(
                io["out"].rearrange("(b p) n -> b p n", p=128)[0][:, 0:PAIRS], dummy)

        if x1p_cm is not None:
            x1p_cm.__exit__(None, None, None)


def dram_decls(cfg, causal):
    d = {
        "xt": ([cfg.ne, cfg.sl], BF), "xqt": ([cfg.ne, cfg.toks], BF),
        "xq": ([cfg.toks, cfg.ne], F32),
        "qw": ([cfg.ne, cfg.ne], BF), "kw": ([cfg.ne, cfg.ne], BF),
        "vw": ([cfg.ne, cfg.ne], BF),
        "qb": ([128, cfg.pairs], F32), "kb": ([128, cfg.pairs], F32),
        "vb": ([128, cfg.ne], F32),
        "ow": ([cfg.ne, cfg.ne], BF),
        "w1p": ([cfg.fch, cfg.ne, 128], BF), "b1c": ([128, cfg.fch], F32),
        "w2": ([cfg.nhid, cfg.ne], BF),
        "ln1a": ([128, cfg.ne], F32), "ln1b": ([128, cfg.ne], F32),
        "ln2a": ([128, cfg.ne], F32), "ln2b": ([128, cfg.ne], F32),
    }
    if causal:
        d["amask"] = ([cfg.nb, 128, 128], BF)
    else:
        d["amask_full"] = ([cfg.nb, 128, cfg.toks], F32)
    return d


_NC_CACHE = {}


def build_nc(causal, cfg=FULL, n_cores=8):
    key = (causal, cfg.ne, cfg.sl, cfg.nh, cfg.nhid)
    if key in _NC_CACHE:
        return _NC_CACHE[key]
    nc = bacc.Bacc("TRN2", num_devices=n_cores)
    io = {}
    for name, (shape, dt) in dram_decls(cfg, causal).items():
        io[name] = nc.dram_tensor(name, shape, dt, kind="ExternalInput").ap()
    io["out"] = nc.dram_tensor("out", [cfg.toks, cfg.ne], F32,
                               kind="ExternalOutput").ap()
    with tile.TileContext(nc) as tc:
        emit(tc, cfg, io, causal)
    nc.compile()
    _NC_CACHE[key] = nc
    return nc


def build_amask(par, cfg):
    am = np.ones((cfg.nb, 128, 128), np.float32)
    blocks = blocks_for(par, cfg, True)
    kk = np.arange(128)[:, None]
    qq = np.arange(128)[None, :]
    for t, i_t in enumerate(blocks):
        nj = i_t + 1
        for j in range(2 * t, 2 * t + 2):
            if j >= cfg.nb:
                continue
            if j == nj - 1:
                am[j] = (kk <= qq).astype(np.float32)
            elif j >= nj:
                am[j] = 0.0
    return am.astype(bf16)


def build_amask_full(par, cfg, mask2d):
    am = np.zeros((cfg.nb, 128, cfg.toks), np.float32)
    blocks = blocks_for(par, cfg, False)
    for j in range(cfg.nb):
        for t, i_t in enumerate(blocks):
            blk = mask2d[i_t * 128:(i_t + 1) * 128, j * 128:(j + 1) * 128]
            am[j][:, t * 128:(t + 1) * 128] = np.where(blk.T == 0, NEG, 0.0)
    return am


def prep_core(inputs, core, causal, cfg=FULL):
    b, par = core // 2, core % 2
    blocks = blocks_for(par, cfg, causal)
    ne, pairs, fch = cfg.ne, cfg.pairs, cfg.fch
    x = np.asarray(inputs["x"][b], np.float32)
    tok_idx = np.concatenate([np.arange(i * 128, (i + 1) * 128) for i in blocks])
    qkv_w = np.asarray(inputs["qkv_w"], np.float32)
    qkv_b = np.asarray(inputs["qkv_b"], np.float32)
    bcast = lambda v: np.broadcast_to(np.asarray(v, np.float32), (128, v.shape[0])).copy()
    d = {
        "xt": np.ascontiguousarray(x.T).astype(bf16),
        "xqt": np.ascontiguousarray(x[tok_idx].T).astype(bf16),
        "xq": np.ascontiguousarray(x[tok_idx])
              + np.asarray(inputs["o_b"], np.float32)[None, :],
        "qw": qkv_w[:, :ne].astype(bf16),
        "kw": np.ascontiguousarray(qkv_w[:, ne:2 * ne]).astype(bf16),
        "vw": np.ascontiguousarray(qkv_w[:, 2 * ne:]).astype(bf16),
        "qb": np.ascontiguousarray(qkv_b[:ne].reshape(pairs, 128).T),
        "kb": np.ascontiguousarray(qkv_b[ne:2 * ne].reshape(pairs, 128).T),
        "vb": bcast(qkv_b[2 * ne:]),
        "ow": np.asarray(inputs["o_w"], np.float32).astype(bf16),
        "w1p": np.ascontiguousarray(
            np.asarray(inputs["w1"], np.float32).astype(bf16)
            .reshape(cfg.ne, cfg.fch, 128).transpose(1, 0, 2)),
        # b2 (the FFN2 output bias) is folded into ln1_b so x1 already
        # carries the residual bias; compensate FFN1's bias so
        # relu(x1' @ w1 + b1') == relu(x1 @ w1 + b1).
        "b1c": np.ascontiguousarray(
            (np.asarray(inputs["b1"], np.float32)
             - np.asarray(inputs["b2"], np.float32)
             @ np.asarray(inputs["w1"], np.float32).astype(bf16)
               .astype(np.float32)).reshape(fch, 128).T),
        "w2": np.asarray(inputs["w2"], np.float32).astype(bf16),
        "ln1a": bcast(np.asarray(inputs["ln1_a"], np.float32)),
        "ln1b": bcast(np.asarray(inputs["ln1_b"], np.float32)
                      + np.asarray(inputs["b2"], np.float32)),
        "ln2a": bcast(np.asarray(inputs["ln2_a"], np.float32)),
        "ln2b": bcast(np.asarray(inputs["ln2_b"], np.float32)),
    }
    if causal:
        d["amask"] = build_amask(par, cfg)
    else:
        mask2d = np.asarray(inputs["mask"])[0, 0]
        d["amask_full"] = build_amask_full(par, cfg, mask2d)
    return d


def assemble(results, causal, cfg=FULL):
    out = np.empty((cfg.bs, cfg.sl, cfg.ne), np.float32)
    for core in range(cfg.bs * 2):
        b, par = core // 2, core % 2
        blocks = blocks_for(par, cfg, causal)
        r = results[core]["out"]
        for t, i_t in enumerate(blocks):
            out[b, i_t * 128:(i_t + 1) * 128] = r[t * 128:(t + 1) * 128]
    return out


def is_causal_mask(mask):
    m = np.asarray(mask)[0, 0]
    n = m.shape[0]
    return bool(np.array_equal(m != 0, np.tril(np.ones((n, n), bool))))


def kernel(**inputs):
    cfg = FULL
    causal = is_causal_mask(inputs["mask"])
    nc = build_nc(causal, cfg)
    in_maps = [prep_core(inputs, c, causal, cfg) for c in range(8)]
    res = run_bass_kernel_spmd(nc, in_maps, core_ids=list(range(8)), trace=False)
    return assemble(res.results, causal, cfg)


# revision 69
# speedup vs baseline: 1.1244x; 1.1244x over previous
"""Trainium2 Bass kernel for nn_Block (dense transformer block), 8-core SPMD.

Sharding: core c -> batch c//2, half of the causal q-blocks (interleaved
assignment {i : i%4 in {0,3}} / {i%4 in {1,2}} for exact causal balance).
K/V are computed per-core for the whole batch (duplicated across the 2 cores
of a batch); everything after attention is purely token-parallel, so no
collectives are needed and each core writes its own output rows.

Layout strategy (all matmuls bf16, fp32 accumulation):
  - x^T, K^T, Q^T kept feature-on-partitions so attention scores are computed
    directly transposed: S^T[k,q] = (K^T chunk).T @ Q^T.
  - V carries 64 appended ones-columns (vo rows 64:127), so the O^T
    accumulation matmul broadcasts the softmax denominator across 64 PSUM
    partitions for free; the reciprocal then runs 64-wide on DVE.
  - Attention runs head-major: one [128,TOKS] PSUM accumulator per head.
  - FFN: all of h^T is produced first (bf16, 8MB), then the second linear
    accumulates the full 4096-deep contraction in PSUM per token block (no
    SBUF accumulator round-trips). b2 is folded into ln1_b on the host
    (b1 compensated by -b2 @ w1) so the finalize is one add + LN.
  - LayerNorm uses a fused scalar_tensor_tensor for (xc*rstd)*a and writes
    its output into the xc scratch tile when the caller allows.
"""

import numpy as np
import ml_dtypes

import concourse.bacc as bacc
import concourse.mybir as mybir
import concourse.tile as tile
from concourse.alu_op_type import AluOpType
from concourse.masks import make_identity
from concourse.bass_utils import run_bass_kernel_spmd

BF = mybir.dt.bfloat16
F32 = mybir.dt.float32
AF = mybir.ActivationFunctionType
AX = mybir.AxisListType
bf16 = ml_dtypes.bfloat16

EPS = 1e-5
NEG = -1e30


class Cfg:
    def __init__(self, ne=1024, sl=2048, nh=16, nhid=4096, bs=4):
        self.ne, self.sl, self.nh, self.nhid, self.bs = ne, sl, nh, nhid, bs
        self.dh = 64
        self.e = ne // 128          # feature chunks
        self.nb = sl // 128         # k/token blocks per batch
        self.slots = self.nb // 2   # q-blocks per core
        self.toks = self.slots * 128
        self.pairs = nh // 2
        self.fch = nhid // 128      # ffn feature chunks
        self.scale = self.dh ** -0.5


FULL = Cfg()


def blocks_for(par, cfg, causal):
    if causal:
        keep = (0, 3) if par == 0 else (1, 2)
        return [i for i in range(cfg.nb) if i % 4 in keep]
    return list(range(par * cfg.slots, (par + 1) * cfg.slots))


def chunks(start, end, step=512):
    out = []
    c = start
    while c < end:
        w = min(end, (c // step + 1) * step) - c
        out.append((c, w))
        c += w
    return out


def layer_norm(nc, pool, out_ap, x_ap, a_ap, b_ap, n, tag, eps_ap):
    """out = (x - mean(x)) / (std(x, ddof=1) + EPS) * a + b, rows on
    partitions. If out_ap is None, writes into the xc scratch tile and
    returns that AP (saves a pool buffer for fire-and-forget DMA sources)."""
    st = pool.tile([128, 8], F32, tag=f"{tag}s", name=f"{tag}s")
    nc.vector.reduce_sum(st[:, 0:1], x_ap, axis=AX.X)
    nc.scalar.mul(st[:, 1:2], st[:, 0:1], -1.0 / n)
    xc = pool.tile([128, n], F32, tag=f"{tag}xc", name=f"{tag}xc")
    nc.scalar.add(xc, x_ap, st[:, 1:2])
    tmp = pool.tile([128, n], F32, tag=f"{tag}t", name=f"{tag}t")
    nc.scalar.activation(tmp, xc, AF.Square, accum_out=st[:, 2:3])
    nc.scalar.activation(st[:, 3:4], st[:, 2:3], AF.Sqrt, scale=1.0 / (n - 1))
    nc.scalar.add(st[:, 4:5], st[:, 3:4], eps_ap)
    nc.vector.reciprocal(st[:, 5:6], st[:, 4:5])
    nc.vector.scalar_tensor_tensor(tmp, xc, st[:, 5:6], a_ap,
                                   AluOpType.mult, AluOpType.mult)
    if out_ap is None:
        out_ap = xc
    nc.vector.tensor_add(out_ap, tmp, b_ap)
    return out_ap


def emit(tc, cfg, io, causal, upto="full"):
    nc = tc.nc
    E, NB, SLOTS, PAIRS = cfg.e, cfg.nb, cfg.slots, cfg.pairs
    NE, SL, TOKS, FCH = cfg.ne, cfg.sl, cfg.toks, cfg.fch

    def vk_phase(xtp, qkvp, psk, psv, kt, qt, vo, kb, vb, qb):
        xt = xtp.tile([128, E, SL], BF, tag="xt", name="xt", bufs=1)
        xt_src = io["xt"].rearrange("(e p) t -> p e t", p=128)
        vw = qkvp.tile([128, E, NE], BF, tag="w", name="w")
        vw_src = io["vw"].rearrange("(e p) n -> p e n", p=128)
        # interleave, with a small leading slice of xt so the first V psum
        # chains can start as soon as possible
        for e in range(E):
            nc.sync.dma_start(vw[:, e, :], vw_src[:, e, :])
            nc.sync.dma_start(xt[:, e, 0:256], xt_src[:, e, 0:256])
        for e in range(E):
            nc.sync.dma_start(xt[:, e, 256:], xt_src[:, e, 256:])
        nc.gpsimd.memset(vo[:, :, :, 64:128], 1.0)
        for j in range(NB):
            ps = psv.tile([128, NE], F32, tag="pv", name="pv")
            for (cs, cw) in chunks(0, NE):
                for e in range(E):
                    nc.tensor.matmul(
                        ps[:, cs:cs + cw],
                        lhsT=xt[:, e, j * 128:(j + 1) * 128],
                        rhs=vw[:, e, cs:cs + cw],
                        start=(e == 0), stop=(e == E - 1))
            nc.vector.tensor_add(
                vo[:, j, :, 0:64],
                ps.rearrange("p (h d) -> p h d", d=64),
                vb.rearrange("p (h d) -> p h d", d=64))
        # K^T all pairs
        kw = qkvp.tile([128, E, NE], BF, tag="w", name="w")
        kw_src = io["kw"].rearrange("(e p) n -> p e n", p=128)
        for e in range(E):
            nc.sync.dma_start(kw[:, e, :], kw_src[:, e, :])
        for pair in range(PAIRS):
            for (cs, cw) in chunks(0, SL, 1024):
                ps = psk.tile([128, 1024], F32, tag="pk", name="pk")[:, :cw]
                for (c2, w2) in chunks(0, cw):
                    for e in range(E):
                        nc.tensor.matmul(
                            ps[:, c2:c2 + w2],
                            lhsT=kw[:, e, pair * 128:(pair + 1) * 128],
                            rhs=xt[:, e, cs + c2:cs + c2 + w2],
                            start=(e == 0), stop=(e == E - 1))
                nc.scalar.activation(kt[:, pair, cs:cs + cw], ps,
                                     AF.Identity, bias=kb[:, pair:pair + 1])
    def q_phase(qp, psk, qt, qb):
        qw = qp.tile([128, E, NE], BF, tag="qw", name="qw", bufs=1)
        qw_src = io["qw"].rearrange("(e p) n -> p e n", p=128)
        for e in range(E):
            nc.sync.dma_start(qw[:, e, :], qw_src[:, e, :])
        xqt = qp.tile([128, E, TOKS], BF, tag="xqt", name="xqt", bufs=1)
        xqt_src = io["xqt"].rearrange("(e p) t -> p e t", p=128)
        for e in range(E):
            nc.sync.dma_start(xqt[:, e, :], xqt_src[:, e, :])
        for pair in range(PAIRS):
            ps = psk.tile([128, 1024], F32, tag="pk", name="pk")[:, :TOKS]
            for (cs, cw) in chunks(0, TOKS):
                for e in range(E):
                    nc.tensor.matmul(
                        ps[:, cs:cs + cw],
                        lhsT=qw[:, e, pair * 128:(pair + 1) * 128],
                        rhs=xqt[:, e, cs:cs + cw],
                        start=(e == 0), stop=(e == E - 1))
            nc.vector.tensor_scalar_add(qt[:, pair, :], ps,
                                        qb[:, pair:pair + 1])

    def att_head(attp, psa1, psa2, kt, qt, vo, yt, am, pair, base):
        head = 2 * pair + (base >> 6)
        po = psa1.tile([128, TOKS], F32, tag="pso", name="pso")
        pts = {}

        def emit_scores(j):
            c0 = (j // 2) * 128 if causal else 0
            pt = attp.tile([128, TOKS], BF, tag="pt", name="pt", bufs=6)
            pts[j] = pt
            pss = psa2.tile([128, TOKS], F32, tag="pss",
                            name="pss")[:, :TOKS - c0]
            for (cs, cw) in chunks(0, TOKS - c0):
                nc.tensor.matmul(
                    pss[:, cs:cs + cw],
                    lhsT=kt[base:base + 64, pair, j * 128:(j + 1) * 128],
                    rhs=qt[base:base + 64, pair, c0 + cs:c0 + cs + cw],
                    start=True, stop=True)
            if not causal:
                amj = attp.tile([128, TOKS], F32, tag="amj", name="amj")
                nc.sync.dma_start(
                    amj, io["amask_full"].rearrange("j p q -> p j q")[:, j, :])
                nc.vector.tensor_add(pss, pss, amj)
            nc.scalar.activation(pt[:, c0:], pss, AF.Exp, scale=cfg.scale)
            if causal:
                nc.vector.tensor_mul(
                    pt[:, c0:c0 + 128], pt[:, c0:c0 + 128], am[:, j, :])

        def emit_av(j):
            c0 = (j // 2) * 128 if causal else 0
            pt = pts.pop(j)
            for (cs, cw) in chunks(c0, TOKS):
                if causal:
                    stop_j = 2 * (min((cs // 512 + 1) * 4, SLOTS) - 1) + 1
                else:
                    stop_j = NB - 1
                nc.tensor.matmul(
                    po[:, cs:cs + cw], lhsT=vo[:, j, head, :],
                    rhs=pt[:, cs:cs + cw], start=(j == 0),
                    stop=(j == stop_j))

        # engines run their queues in program order, so software-pipeline the
        # emission: scores run SKEW iterations ahead of the AV consumer, and
        # PE never sits behind an AV waiting for its exp
        SKEW = 3
        for j in range(NB + SKEW):
            if j < NB:
                emit_scores(j)
            if j - SKEW >= 0:
                emit_av(j - SKEW)
        # vo rows 64..127 are all-ones, so po[64:128] holds the softmax
        # denominator already broadcast across 64 partitions
        rbi = attp.tile([64, TOKS], F32, tag="rbi", name="rbi")
        nc.vector.reciprocal(rbi, po[64:128, :])
        if base == 0:
            nc.vector.tensor_mul(yt[0:64, pair, :], po[0:64, :], rbi)
        else:
            ystg = attp.tile([64, TOKS], BF, tag="ystg", name="ystg")
            nc.vector.tensor_mul(ystg, po[0:64, :], rbi)
            nc.sync.dma_start(yt[64:128, pair, :], ystg)

    def oproj_phase(op, pso_p, ptr_p, yt, x1, x1t, ident, eps_ap):
        ow = op.tile([128, E, NE], BF, tag="ow", name="ow", bufs=1)
        ow_src = io["ow"].rearrange("(f p) n -> p f n", p=128)
        for f in range(E):
            nc.sync.dma_start(ow[:, f, :], ow_src[:, f, :])
        ln1a = op.tile([128, NE], F32, tag="ln1a", name="ln1a", bufs=1)
        nc.sync.dma_start(ln1a, io["ln1a"])
        ln1b = op.tile([128, NE], F32, tag="ln1b", name="ln1b", bufs=1)
        nc.sync.dma_start(ln1b, io["ln1b"])
        xq_src = io["xq"].rearrange("(b p) n -> b p n", p=128)
        # transposes for tb are emitted two iterations late so PE does not
        # stall behind tb's LayerNorm before starting tb+1's o_proj matmuls
        for tb in range(SLOTS + 3):
            if tb < SLOTS:
                ps = pso_p.tile([128, NE], F32, tag="po", name="po")
                for (cs, cw) in chunks(0, NE):
                    for f in range(E):
                        nc.tensor.matmul(
                            ps[:, cs:cs + cw],
                            lhsT=yt[:, f, tb * 128:(tb + 1) * 128],
                            rhs=ow[:, f, cs:cs + cw],
                            start=(f == 0), stop=(f == E - 1))
                xq_t = op.tile([128, NE], F32, tag="xq", name="xq")
                nc.sync.dma_start(xq_t, xq_src[tb])
                t2 = op.tile([128, NE], F32, tag="t2", name="t2")
                nc.vector.tensor_add(t2, ps, xq_t)
                layer_norm(nc, op, x1[:, tb, :], t2, ln1a, ln1b, NE, "ln1",
                           eps_ap)
            if tb >= 3:
                for e in range(E):
                    ptr = ptr_p.tile([128, 128], BF, tag="ptr", name="ptr",
                                     bufs=2)
                    nc.tensor.transpose(
                        ptr, x1[:, tb - 3, e * 128:(e + 1) * 128], ident)
                    nc.scalar.copy(
                        x1t[:, e, (tb - 3) * 128:(tb - 2) * 128], ptr)

    def ffn_phase(fp, x1, x1t, eps_ap):
        b1c = fp.tile([128, FCH], F32, tag="b1c", name="b1c", bufs=1)
        nc.sync.dma_start(b1c, io["b1c"])
        ln2a = fp.tile([128, NE], F32, tag="ln2a", name="ln2a", bufs=1)
        nc.sync.dma_start(ln2a, io["ln2a"])
        ln2b = fp.tile([128, NE], F32, tag="ln2b", name="ln2b", bufs=1)
        nc.sync.dma_start(ln2b, io["ln2b"])
        w2t = fp.tile([128, FCH, NE], BF, tag="w2t", name="w2t", bufs=1)
        w2_src = io["w2"].rearrange("(f p) n -> p f n", p=128)
        ht = fp.tile([128, FCH, TOKS], BF, tag="ht", name="ht", bufs=1)
        with tc.tile_pool(name="psff1", bufs=3, space="PSUM") as ps1:
            for f in range(FCH):
                w1f = fp.tile([128, E, 128], BF, tag="w1f", name="w1f", bufs=3)
                nc.sync.dma_start(
                    w1f, io["w1p"][f].rearrange("(e p) q -> p e q", p=128))
                nc.sync.dma_start(w2t[:, f, :], w2_src[:, f, :])
                psh = ps1.tile([128, TOKS], F32, tag="psh", name="psh")
                for (cs, cw) in chunks(0, TOKS):
                    for e in range(E):
                        nc.tensor.matmul(
                            psh[:, cs:cs + cw], lhsT=w1f[:, e, :],
                            rhs=x1t[:, e, cs:cs + cw],
                            start=(e == 0), stop=(e == E - 1))
                nc.scalar.activation(ht[:, f, :], psh,
                                     AF.Relu, bias=b1c[:, f:f + 1])
        out_dst = io["out"].rearrange("(b p) n -> b p n", p=128)
        with tc.tile_pool(name="psff2", bufs=3, space="PSUM") as ps2:
            for tb in range(SLOTS):
                psF = ps2.tile([128, NE], F32, tag="psF", name="psF")
                for fi in range(FCH):
                    for (cs, cw) in chunks(0, NE):
                        nc.tensor.matmul(
                            psF[:, cs:cs + cw],
                            lhsT=ht[:, fi, tb * 128:(tb + 1) * 128],
                            rhs=w2t[:, fi, cs:cs + cw],
                            start=(fi == 0), stop=(fi == FCH - 1))
                t1 = fp.tile([128, NE], F32, tag="ft1", name="ft1")
                nc.vector.tensor_add(t1, psF, x1[:, tb, :])
                outt = layer_norm(nc, fp, None, t1, ln2a, ln2b, NE, "ln2",
                                  eps_ap)
                nc.sync.dma_start(out_dst[tb], outt)

    with tc.tile_pool(name="const", bufs=1) as constp:
        ident = constp.tile([128, 128], BF, tag="ident", name="ident")
        make_identity(nc, ident)
        eps_ap = constp.tile([128, 1], F32, tag="eps", name="eps")
        nc.vector.memset(eps_ap, EPS)
        qb = constp.tile([128, PAIRS], F32, tag="qb", name="qb")
        nc.sync.dma_start(qb, io["qb"])
        kb = constp.tile([128, PAIRS], F32, tag="kb", name="kb")
        nc.sync.dma_start(kb, io["kb"])
        vb = constp.tile([128, NE], F32, tag="vb", name="vb")
        nc.sync.dma_start(vb, io["vb"])
        am = None
        if causal:
            am = constp.tile([128, NB, 128], BF, tag="am", name="am")
            nc.sync.dma_start(am, io["amask"].rearrange("j p q -> p j q"))

        ytp_cm = tc.tile_pool(name="ytp", bufs=1)
        ytp = ytp_cm.__enter__()
        yt = ytp.tile([128, PAIRS, TOKS], BF, tag="yt", name="yt")

        with tc.tile_pool(name="kqvo", bufs=1) as kqvo:
            kt = kqvo.tile([128, PAIRS, SL], BF, tag="kt", name="kt")
            qt = kqvo.tile([128, PAIRS, TOKS], BF, tag="qt", name="qt")
            vo = kqvo.tile([128, NB, cfg.nh, 128], BF, tag="vo", name="vo")
            with tc.tile_pool(name="psk", bufs=2, space="PSUM") as psk:
                with (
                    tc.tile_pool(name="qkv", bufs=2) as qkvp,
                    tc.tile_pool(name="xtp", bufs=1) as xtp,
                    tc.tile_pool(name="psv", bufs=2, space="PSUM") as psv,
                ):
                    vk_phase(xtp, qkvp, psk, psv, kt, qt, vo, kb, vb, qb)
                with tc.tile_pool(name="qp", bufs=1) as qp:
                    q_phase(qp, psk, qt, qb)
            if upto != "qkv":
                with (
                    tc.tile_pool(name="att", bufs=3) as attp,
                    tc.tile_pool(name="psatt1", bufs=1, space="PSUM") as psa1,
                    tc.tile_pool(name="psatt2", bufs=3, space="PSUM") as psa2,
                ):
                    for pair in range(PAIRS):
                        for base in (0, 64):
                            att_head(attp, psa1, psa2, kt, qt, vo, yt, am,
                                     pair, base)

        x1p_cm = None
        if upto in ("oproj", "full"):
            x1p_cm = tc.tile_pool(name="x1p", bufs=1, side="right")
            x1p = x1p_cm.__enter__()
            x1 = x1p.tile([128, SLOTS, NE], BF, tag="x1", name="x1")
            x1t = x1p.tile([128, E, TOKS], BF, tag="x1t", name="x1t")
            with (
                tc.tile_pool(name="oproj", bufs=3) as op,
                tc.tile_pool(name="psop", bufs=3, space="PSUM") as pso_p,
                tc.tile_pool(name="psoptr", bufs=1, space="PSUM") as ptr_p,
            ):
                oproj_phase(op, pso_p, ptr_p, yt, x1, x1t, ident, eps_ap)

        ytp_cm.__exit__(None, None, None)

        if upto == "full":
            with tc.tile_pool(name="ffn", bufs=2) as fp:
                ffn_phase(fp, x1, x1t, eps_ap)
        else:
            dummy = constp.tile([128, PAIRS], F32, tag="dummy", name="dummy")
            nc.vector.tensor_copy(dummy, qb)
            nc.sync.dma_start(
                io["out"].rearrange("(b p) n -> b p n", p=128)[0][:, 0:PAIRS], dummy)

        if x1p_cm is not None:
            x1p_cm.__exit__(None, None, None)


def dram_decls(cfg, causal):
    d = {
        "xt": ([cfg.ne, cfg.sl], BF), "xqt": ([cfg.ne, cfg.toks], BF),
        "xq": ([cfg.toks, cfg.ne], F32),
        "qw": ([cfg.ne, cfg.ne], BF), "kw": ([cfg.ne, cfg.ne], BF),
        "vw": ([cfg.ne, cfg.ne], BF),
        "qb": ([128, cfg.pairs], F32), "kb": ([128, cfg.pairs], F32),
        "vb": ([128, cfg.ne], F32),
        "ow": ([cfg.ne, cfg.ne], BF),
        "w1p": ([cfg.fch, cfg.ne, 128], BF), "b1c": ([128, cfg.fch], F32),
        "w2": ([cfg.nhid, cfg.ne], BF),
        "ln1a": ([128, cfg.ne], F32), "ln1b": ([128, cfg.ne], F32),
        "ln2a": ([128, cfg.ne], F32), "ln2b": ([128, cfg.ne], F32),
    }
    if causal:
        d["amask"] = ([cfg.nb, 128, 128], BF)
    else:
        d["amask_full"] = ([cfg.nb, 128, cfg.toks], F32)
    return d


_NC_CACHE = {}


def build_nc(causal, cfg=FULL, n_cores=8):
    key = (causal, cfg.ne, cfg.sl, cfg.nh, cfg.nhid)
    if key in _NC_CACHE:
        return _NC_CACHE[key]
    nc = bacc.Bacc("TRN2", num_devices=n_cores)
    io = {}
    for name, (shape, dt) in dram_decls(cfg, causal).items():
        io[name] = nc.dram_tensor(name, shape, dt, kind="ExternalInput").ap()
    io["out"] = nc.dram_tensor("out", [cfg.toks, cfg.ne], F32,
                               kind="ExternalOutput").ap()
    with tile.TileContext(nc) as tc:
        emit(tc, cfg, io, causal)
    nc.compile()
    _NC_CACHE[key] = nc
    return nc


def build_amask(par, cfg):
    am = np.ones((cfg.nb, 128, 128), np.float32)
    blocks = blocks_for(par, cfg, True)
    kk = np.arange(128)[:, None]
    qq = np.arange(128)[None, :]
    for t, i_t in enumerate(blocks):
        nj = i_t + 1
        for j in range(2 * t, 2 * t + 2):
            if j >= cfg.nb:
                continue
            if j == nj - 1:
                am[j] = (kk <= qq).astype(np.float32)
            elif j >= nj:
                am[j] = 0.0
    return am.astype(bf16)


def build_amask_full(par, cfg, mask2d):
    am = np.zeros((cfg.nb, 128, cfg.toks), np.float32)
    blocks = blocks_for(par, cfg, False)
    for j in range(cfg.nb):
        for t, i_t in enumerate(blocks):
            blk = mask2d[i_t * 128:(i_t + 1) * 128, j * 128:(j + 1) * 128]
            am[j][:, t * 128:(t + 1) * 128] = np.where(blk.T == 0, NEG, 0.0)
    return am


def prep_core(inputs, core, causal, cfg=FULL):
    b, par = core // 2, core % 2
    blocks = blocks_for(par, cfg, causal)
    ne, pairs, fch = cfg.ne, cfg.pairs, cfg.fch
    x = np.asarray(inputs["x"][b], np.float32)
    tok_idx = np.concatenate([np.arange(i * 128, (i + 1) * 128) for i in blocks])
    qkv_w = np.asarray(inputs["qkv_w"], np.float32)
    qkv_b = np.asarray(inputs["qkv_b"], np.float32)
    bcast = lambda v: np.broadcast_to(np.asarray(v, np.float32), (128, v.shape[0])).copy()
    d = {
        "xt": np.ascontiguousarray(x.T).astype(bf16),
        "xqt": np.ascontiguousarray(x[tok_idx].T).astype(bf16),
        "xq": np.ascontiguousarray(x[tok_idx])
              + np.asarray(inputs["o_b"], np.float32)[None, :],
        "qw": qkv_w[:, :ne].astype(bf16),
        "kw": np.ascontiguousarray(qkv_w[:, ne:2 * ne]).astype(bf16),
        "vw": np.ascontiguousarray(qkv_w[:, 2 * ne:]).astype(bf16),
        "qb": np.ascontiguousarray(qkv_b[:ne].reshape(pairs, 128).T),
        "kb": np.ascontiguousarray(qkv_b[ne:2 * ne].reshape(pairs, 128).T),
        "vb": bcast(qkv_b[2 * ne:]),
        "ow": np.asarray(inputs["o_w"], np.float32).astype(bf16),
        "w1p": np.ascontiguousarray(
            np.asarray(inputs["w1"], np.float32).astype(bf16)
            .reshape(cfg.ne, cfg.fch, 128).transpose(1, 0, 2)),
        # b2 (the FFN2 output bias) is folded into ln1_b so x1 already
        # carries the residual bias; compensate FFN1's bias so
        # relu(x1' @ w1 + b1') == relu(x1 @ w1 + b1).
        "b1c": np.ascontiguousarray(
            (np.asarray(inputs["b1"], np.float32)
             - np.asarray(inputs["b2"], np.float32)
             @ np.asarray(inputs["w1"], np.float32).astype(bf16)
               .astype(np.float32)).reshape(fch, 128).T),
        "w2": np.asarray(inputs["w2"], np.float32).astype(bf16),
        "ln1a": bcast(np.asarray(inputs["ln1_a"], np.float32)),
        "ln1b": bcast(np.asarray(inputs["ln1_b"], np.float32)
                      + np.asarray(inputs["b2"], np.float32)),
        "ln2a": bcast(np.asarray(inputs["ln2_a"], np.float32)),
        "ln2b": bcast(np.asarray(inputs["ln2_b"], np.float32)),
    }
    if causal:
        d["amask"] = build_amask(par, cfg)
    else:
        mask2d = np.asarray(inputs["mask"])[0, 0]
        d["amask_full"] = build_amask_full(par, cfg, mask2d)
    return d


def assemble(results, causal, cfg=FULL):
    out = np.empty((cfg.bs, cfg.sl, cfg.ne), np.float32)
    for core in range(cfg.bs * 2):
        b, par = core // 2, core % 2
        blocks = blocks_for(par, cfg, causal)
        r = results[core]["out"]
        for t, i_t in enumerate(blocks):
            out[b, i_t * 128:(i_t + 1) * 128] = r[t * 128:(t + 1) * 128]
    return out


def is_causal_mask(mask):
    m = np.asarray(mask)[0, 0]
    n = m.shape[0]
    return bool(np.array_equal(m != 0, np.tril(np.ones((n, n), bool))))


def kernel(**inputs):
    cfg = FULL
    causal = is_causal_mask(inputs["mask"])
    nc = build_nc(causal, cfg)
    in_maps = [prep_core(inputs, c, causal, cfg) for c in range(8)]
    res = run_bass_kernel_spmd(nc, in_maps, core_ids=list(range(8)), trace=False)
    return assemble(res.results, causal, cfg)
